# revision 1
# baseline (speedup 1.0000x reference)
"""Trainium2 kernel for BinaryLinear: out = x @ sign(clip(weight,-1,1)).T + bias.

Full shapes: x [8192, 4096] f32, weight [4096, 4096] f32, bias [4096] f32,
out [8192, 4096] f32.

Strategy (8 NeuronCores, no collectives needed):
  - Grid-shard tokens x out_features across the 8 cores; each core computes
    a disjoint output tile, host slices inputs / stitches outputs.
  - Binarized weights are exactly +-1 (bf16/f32r-exact). The matmul runs
    on the PE at 1 cycle/row using float32r operands (f32 bits, reduced-
    precision multiplier, ~2^-13 per-term error -> ~1e-4 rel overall).
  - Host packs x transposed+tiled so the contraction dim (in_features)
    lands on SBUF partitions with every DMA contiguous at line rate.
  - Per core: resident binarized-transposed weight slice in SBUF,
    stream 128-token blocks of xT, accumulate over K=4096 in PSUM,
    add bias on DVE while copying PSUM->SBUF, DMA out.

MODE:
  "f32r"  : single pass, f32r x f32r, 2x4 grid (tok x outf). ~1e-4 rel.
  "bf16x2": x split hi/lo into two bf16 passes, 4x2 grid. ~2e-6 rel,
            about 1.8x slower.
"""

import sys

if "/opt/trn_rl_repo" not in sys.path:
    sys.path.insert(0, "/opt/trn_rl_repo")

import ml_dtypes
import numpy as np

MODE = "f32r"

N_TOK, D_IN, D_OUT = 8192, 4096, 4096
if MODE == "f32r":
    TOK_SHARDS, OUT_SHARDS = 2, 4
else:
    TOK_SHARDS, OUT_SHARDS = 4, 2
N_CORES = TOK_SHARDS * OUT_SHARDS
TOK_C = N_TOK // TOK_SHARDS
OUT_C = D_OUT // OUT_SHARDS
MB = TOK_C // 128  # token blocks per core
KB = D_IN // 128  # contraction blocks
NF = 512  # matmul moving free dim (one fp32 PSUM bank)
NB = OUT_C // NF  # PSUM banks per token block

_cached_nc = None


def build_nc():
    import concourse.bacc as bacc
    import concourse.mybir as mybir
    import concourse.tile as tile

    dt = mybir.dt
    split = MODE == "bf16x2"
    mdt = dt.bfloat16 if split else dt.float32r

    nc = bacc.Bacc()
    xh_d = nc.dram_tensor("xh", [MB, 128, D_IN], mdt, kind="ExternalInput")
    if split:
        xl_d = nc.dram_tensor("xl", [MB, 128, D_IN], mdt, kind="ExternalInput")
    # weights always ship as bf16 (+-1 is exact); the f32r path upconverts
    # on-chip (DVE) so the weight prefetch moves half the bytes.
    wt_d = nc.dram_tensor("wt", [KB, 128, OUT_C], dt.bfloat16, kind="ExternalInput")
    br_d = nc.dram_tensor("br", [128, OUT_C], dt.float32, kind="ExternalInput")
    out_d = nc.dram_tensor("out", [TOK_C, OUT_C], dt.float32, kind="ExternalOutput")

    # First TRICKLE token-blocks are loaded before the weight stream and
    # their matmuls interleaved per k-block, so the PE computes while
    # weights arrive instead of idling at kernel start.
    TRICKLE = 0 if split else 4

    with tile.TileContext(nc) as tc:
        with (
            tc.tile_pool(name="wts", bufs=1) as wpool,
            tc.tile_pool(name="wstage", bufs=2) as spool,
            tc.tile_pool(name="bias", bufs=1) as bpool,
            tc.tile_pool(name="xin", bufs=max(2, TRICKLE)) as xpool,
            tc.tile_pool(name="outp", bufs=1 if not split else 2) as opool,
            tc.tile_pool(name="psum", bufs=8, space="PSUM") as ppool,
        ):

            def load_x(m):
                xh_m = xpool.tile([128, D_IN], mdt, name=f"xh_{m}", tag="xh")
                nc.sync.dma_start(xh_m[:], xh_d[m])
                passes = [xh_m]
                if split:
                    xl_m = xpool.tile([128, D_IN], mdt, name=f"xl_{m}", tag="xl")
                    nc.sync.dma_start(xl_m[:], xl_d[m])
                    passes.append(xl_m)
                return passes

            def alloc_ps(m):
                return [
                    ppool.tile([128, NF], dt.float32, name=f"ps_{m}_{n}", tag="ps")
                    for n in range(NB)
                ]

            def emit_mms(kb, passes, ps):
                n_half = len(passes)
                for hi, xm in enumerate(passes):
                    lhs = xm[:, kb * 128 : (kb + 1) * 128]
                    for n in range(NB):
                        rhs = wts[kb][:, n * NF : (n + 1) * NF]
                        nc.tensor.matmul(
                            ps[n][:],
                            lhs,
                            rhs,
                            start=(kb == 0 and hi == 0),
                            stop=(kb == KB - 1 and hi == n_half - 1),
                        )

            def flush(m, ps):
                out_t = opool.tile([128, OUT_C], dt.float32, name=f"o_{m}", tag="out")
                for n in range(NB):
                    nc.vector.tensor_tensor(
                        out_t[:, n * NF : (n + 1) * NF],
                        ps[n][:],
                        bias_s[:, n * NF : (n + 1) * NF],
                        mybir.AluOpType.add,
                    )
                nc.sync.dma_start(out_d[m * 128 : (m + 1) * 128, :], out_t[:])

            def load_w(kb):
                if split:
                    w = wpool.tile([128, OUT_C], mdt, name=f"wt{kb}", tag=f"wt{kb}")
                    nc.sync.dma_start(w[:], wt_d[kb])
                else:
                    # bf16 DMA + DVE upconvert; matmul bitcasts to f32r
                    stage = spool.tile(
                        [128, OUT_C], dt.bfloat16, name=f"ws{kb}", tag="wstage"
                    )
                    nc.sync.dma_start(stage[:], wt_d[kb])
                    w = wpool.tile(
                        [128, OUT_C], dt.float32r, name=f"wt{kb}", tag=f"wt{kb}"
                    )
                    nc.vector.tensor_copy(w[:], stage[:])
                wts.append(w)

            # Interleave trickle-x loads with the weight stream so both the
            # PE's first operands and the early k-blocks arrive ASAP.
            wts = []
            trickle_x = {}
            if TRICKLE:
                trickle_x[0] = load_x(0)
                for kb in range(0, 6):
                    load_w(kb)
                trickle_x[1] = load_x(1)
                for kb in range(6, 14):
                    load_w(kb)
                trickle_x[2] = load_x(2)
                for kb in range(14, KB):
                    load_w(kb)
            else:
                for kb in range(KB):
                    load_w(kb)
            bias_s = bpool.tile([128, OUT_C], dt.float32, name="bias_s")
            nc.sync.dma_start(bias_s[:], br_d[:])
            for m in range(3, TRICKLE):
                trickle_x[m] = load_x(m)

            if TRICKLE:
                trickle_ps = {m: alloc_ps(m) for m in range(TRICKLE)}
                # m-major kb-chunks ordered to match DMA arrivals of
                # (xt_m, wt[kb]) so the PE never waits on a late tile.
                sched = [
                    (0, 0, 6),
                    (1, 0, 6),
                    (0, 6, 14),
                    (1, 6, 14),
                    (2, 0, 14),
                    (0, 14, KB),
                    (1, 14, KB),
                    (2, 14, KB),
                ] + [(m, 0, KB) for m in range(3, TRICKLE)]
                for m, k0, k1 in sched:
                    for kb in range(k0, k1):
                        emit_mms(kb, trickle_x[m], trickle_ps[m])
                for m in range(TRICKLE):
                    flush(m, trickle_ps[m])

            for m in range(TRICKLE, MB):
                passes = load_x(m)
                ps = alloc_ps(m)
                for kb in range(KB):
                    emit_mms(kb, passes, ps)
                flush(m, ps)

    nc.compile()
    return nc


def _pack_x(a):
    """[TOK_C, D_IN] -> [MB, 128, D_IN] with layout [m, p, (kb t)]:
    packed[m, p, kb*128 + t] = a[m*128 + t, kb*128 + p]."""
    return np.ascontiguousarray(
        a.reshape(MB, 128, KB, 128).transpose(0, 3, 2, 1)
    ).reshape(MB, 128, D_IN)


def prepare_in_maps(x, weight, bias):
    x = np.asarray(x, dtype=np.float32)
    weight = np.asarray(weight, dtype=np.float32)
    bias = np.asarray(bias, dtype=np.float32)
    split = MODE == "bf16x2"
    npdt = ml_dtypes.bfloat16 if split else np.float32

    bw = np.where(weight >= 0, np.float32(1.0), np.float32(-1.0))

    wt_packs, bias_packs = [], []
    for oi in range(OUT_SHARDS):
        w_sh = bw[oi * OUT_C : (oi + 1) * OUT_C]  # [OUT_C, D_IN]
        wt = np.ascontiguousarray(w_sh.T).astype(ml_dtypes.bfloat16)
        wt_packs.append(wt.reshape(KB, 128, OUT_C))
        bias_packs.append(
            np.ascontiguousarray(
                np.broadcast_to(bias[oi * OUT_C : (oi + 1) * OUT_C], (128, OUT_C))
            )
        )

    xh_packs, xl_packs = [], []
    for ti in range(TOK_SHARDS):
        x_sh = x[ti * TOK_C : (ti + 1) * TOK_C]
        if split:
            xh = x_sh.astype(ml_dtypes.bfloat16)
            xh_packs.append(_pack_x(xh))
            xl = (x_sh - xh.astype(np.float32)).astype(ml_dtypes.bfloat16)
            xl_packs.append(_pack_x(xl))
        else:
            xh_packs.append(_pack_x(x_sh))

    in_maps = []
    for c in range(N_CORES):
        ti, oi = divmod(c, OUT_SHARDS)
        m = {"xh": xh_packs[ti], "wt": wt_packs[oi], "br": bias_packs[oi]}
        if split:
            m["xl"] = xl_packs[ti]
        in_maps.append(m)
    return in_maps


def run(in_maps, trace=False, **kwargs):
    global _cached_nc
    from concourse.bass_utils import run_bass_kernel_spmd

    if _cached_nc is None:
        _cached_nc = build_nc()
    return run_bass_kernel_spmd(
        _cached_nc, in_maps, list(range(N_CORES)), trace=trace, **kwargs
    )


def gather(results):
    out = np.empty((N_TOK, D_OUT), dtype=np.float32)
    for c in range(N_CORES):
        ti, oi = divmod(c, OUT_SHARDS)
        out[ti * TOK_C : (ti + 1) * TOK_C, oi * OUT_C : (oi + 1) * OUT_C] = results[c][
            "out"
        ]
    return out


def kernel(x, weight, bias):
    res = run(prepare_in_maps(x, weight, bias), trace=False)
    return gather(res.results)



# revision 2
# speedup vs baseline: 1.1213x; 1.1213x over previous
"""Trainium2 kernel for BinaryLinear: out = x @ sign(clip(weight,-1,1)).T + bias.

Full shapes: x [8192, 4096] f32, weight [4096, 4096] f32, bias [4096] f32,
out [8192, 4096] f32.

Strategy (8 NeuronCores, no collectives needed):
  - Grid-shard tokens x out_features across the 8 cores; each core computes
    a disjoint output tile, host slices inputs / stitches outputs.
  - Both operands ship as fp16: binarized weights are exactly +-1 (fp16-
    exact) and fp16 x contributes ~2^-11 per-term error -> ~1.6e-4 rel
    overall, far inside the 2e-2 gate.
  - fp16 vs f32r: the PE streams both at 1 cycle/row, but the f32r
    stationary pays a 4-byte LDWEIGHTS (~226 ns, can't hide behind a
    213 ns matmul). A 2-byte stationary is FWL-eligible (~53 ns) and
    hides completely in the weight shadow buffer -> ~437 us PE floor.
  - Host packs x transposed+tiled so the contraction dim (in_features)
    lands on SBUF partitions with every DMA contiguous at line rate.
  - Per core: resident binarized-transposed weight slice in SBUF,
    stream 128-token blocks of xT, accumulate over K=4096 in PSUM,
    add bias on DVE while copying PSUM->SBUF, DMA out.
"""

import sys

if "/opt/trn_rl_repo" not in sys.path:
    sys.path.insert(0, "/opt/trn_rl_repo")

import numpy as np

N_TOK, D_IN, D_OUT = 8192, 4096, 4096
TOK_SHARDS, OUT_SHARDS = 2, 4
N_CORES = TOK_SHARDS * OUT_SHARDS
TOK_C = N_TOK // TOK_SHARDS
OUT_C = D_OUT // OUT_SHARDS
MB = TOK_C // 128  # token blocks per core
KB = D_IN // 128  # contraction blocks
NF = 512  # matmul moving free dim (one fp32 PSUM bank)
NB = OUT_C // NF  # PSUM banks per token block

_cached_nc = None


def build_nc():
    import concourse.bacc as bacc
    import concourse.mybir as mybir
    import concourse.tile as tile

    dt = mybir.dt
    mdt = dt.float16

    nc = bacc.Bacc()
    xh_d = nc.dram_tensor("xh", [MB, 128, D_IN], mdt, kind="ExternalInput")
    wt_d = nc.dram_tensor("wt", [KB, 128, OUT_C], mdt, kind="ExternalInput")
    br_d = nc.dram_tensor("br", [128, OUT_C], dt.float32, kind="ExternalInput")
    out_d = nc.dram_tensor("out", [TOK_C, OUT_C], dt.float32, kind="ExternalOutput")

    # First TRICKLE token-blocks are loaded before the weight stream and
    # their matmuls interleaved per k-block, so the PE computes while
    # weights arrive instead of idling at kernel start.
    TRICKLE = 4

    with tile.TileContext(nc) as tc:
        with (
            tc.tile_pool(name="wts", bufs=1) as wpool,
            tc.tile_pool(name="bias", bufs=1) as bpool,
            tc.tile_pool(name="xin", bufs=max(2, TRICKLE)) as xpool,
            tc.tile_pool(name="outp", bufs=2) as opool,
            tc.tile_pool(name="psum", bufs=8, space="PSUM") as ppool,
        ):

            def load_x(m):
                xh_m = xpool.tile([128, D_IN], mdt, name=f"xh_{m}", tag="xh")
                nc.sync.dma_start(xh_m[:], xh_d[m])
                return xh_m

            def alloc_ps(m):
                return [
                    ppool.tile([128, NF], dt.float32, name=f"ps_{m}_{n}", tag="ps")
                    for n in range(NB)
                ]

            def emit_mms(kb, xm, ps):
                lhs = xm[:, kb * 128 : (kb + 1) * 128]
                for n in range(NB):
                    rhs = wts[kb][:, n * NF : (n + 1) * NF]
                    nc.tensor.matmul(
                        ps[n][:],
                        lhs,
                        rhs,
                        start=(kb == 0),
                        stop=(kb == KB - 1),
                    )

            def flush(m, ps):
                out_t = opool.tile([128, OUT_C], dt.float32, name=f"o_{m}", tag="out")
                for n in range(NB):
                    nc.vector.tensor_tensor(
                        out_t[:, n * NF : (n + 1) * NF],
                        ps[n][:],
                        bias_s[:, n * NF : (n + 1) * NF],
                        mybir.AluOpType.add,
                    )
                nc.sync.dma_start(out_d[m * 128 : (m + 1) * 128, :], out_t[:])

            def load_w(kb):
                w = wpool.tile([128, OUT_C], mdt, name=f"wt{kb}", tag=f"wt{kb}")
                nc.sync.dma_start(w[:], wt_d[kb])
                wts.append(w)

            # Interleave trickle-x loads with the weight stream so both the
            # PE's first operands and the early k-blocks arrive ASAP.
            wts = []
            trickle_x = {}
            trickle_x[0] = load_x(0)
            for kb in range(0, 6):
                load_w(kb)
            trickle_x[1] = load_x(1)
            for kb in range(6, 14):
                load_w(kb)
            trickle_x[2] = load_x(2)
            for kb in range(14, KB):
                load_w(kb)
            bias_s = bpool.tile([128, OUT_C], dt.float32, name="bias_s")
            nc.sync.dma_start(bias_s[:], br_d[:])
            for m in range(3, TRICKLE):
                trickle_x[m] = load_x(m)

            trickle_ps = {m: alloc_ps(m) for m in range(TRICKLE)}
            # m-major kb-chunks ordered to match DMA arrivals of
            # (xt_m, wt[kb]) so the PE never waits on a late tile.
            sched = [
                (0, 0, 6),
                (1, 0, 6),
                (0, 6, 14),
                (1, 6, 14),
                (2, 0, 14),
                (0, 14, KB),
                (1, 14, KB),
                (2, 14, KB),
            ] + [(m, 0, KB) for m in range(3, TRICKLE)]
            for m, k0, k1 in sched:
                for kb in range(k0, k1):
                    emit_mms(kb, trickle_x[m], trickle_ps[m])
            for m in range(TRICKLE):
                flush(m, trickle_ps[m])

            for m in range(TRICKLE, MB):
                xm = load_x(m)
                ps = alloc_ps(m)
                for kb in range(KB):
                    emit_mms(kb, xm, ps)
                flush(m, ps)

    nc.compile()
    return nc


def _pack_x(a):
    """[TOK_C, D_IN] -> [MB, 128, D_IN] with layout [m, p, (kb t)]:
    packed[m, p, kb*128 + t] = a[m*128 + t, kb*128 + p]."""
    return np.ascontiguousarray(
        a.reshape(MB, 128, KB, 128).transpose(0, 3, 2, 1)
    ).reshape(MB, 128, D_IN)


def prepare_in_maps(x, weight, bias):
    x = np.asarray(x, dtype=np.float32)
    weight = np.asarray(weight, dtype=np.float32)
    bias = np.asarray(bias, dtype=np.float32)

    bw = np.where(weight >= 0, np.float16(1.0), np.float16(-1.0))

    wt_packs, bias_packs = [], []
    for oi in range(OUT_SHARDS):
        w_sh = bw[oi * OUT_C : (oi + 1) * OUT_C]  # [OUT_C, D_IN]
        wt = np.ascontiguousarray(w_sh.T)
        wt_packs.append(wt.reshape(KB, 128, OUT_C))
        bias_packs.append(
            np.ascontiguousarray(
                np.broadcast_to(bias[oi * OUT_C : (oi + 1) * OUT_C], (128, OUT_C))
            )
        )

    xh_packs = []
    for ti in range(TOK_SHARDS):
        x_sh = x[ti * TOK_C : (ti + 1) * TOK_C].astype(np.float16)
        xh_packs.append(_pack_x(x_sh))

    in_maps = []
    for c in range(N_CORES):
        ti, oi = divmod(c, OUT_SHARDS)
        m = {"xh": xh_packs[ti], "wt": wt_packs[oi], "br": bias_packs[oi]}
        in_maps.append(m)
    return in_maps


def run(in_maps, trace=False, **kwargs):
    global _cached_nc
    from concourse.bass_utils import run_bass_kernel_spmd

    if _cached_nc is None:
        _cached_nc = build_nc()
    return run_bass_kernel_spmd(
        _cached_nc, in_maps, list(range(N_CORES)), trace=trace, **kwargs
    )


def gather(results):
    out = np.empty((N_TOK, D_OUT), dtype=np.float32)
    for c in range(N_CORES):
        ti, oi = divmod(c, OUT_SHARDS)
        out[ti * TOK_C : (ti + 1) * TOK_C, oi * OUT_C : (oi + 1) * OUT_C] = results[c][
            "out"
        ]
    return out


def kernel(x, weight, bias):
    res = run(prepare_in_maps(x, weight, bias), trace=False)
    return gather(res.results)


# revision 3
# speedup vs baseline: 1.3082x; 1.1667x over previous
"""Trainium2 kernel for BinaryLinear: out = x @ sign(clip(weight,-1,1)).T + bias.

Full shapes: x [8192, 4096] f32, weight [4096, 4096] f32, bias [4096] f32,
out [8192, 4096] f32.

Strategy (8 NeuronCores, no collectives needed):
  - Grid-shard tokens x out_features across the 8 cores (2x4); each core
    computes a disjoint output tile, host slices inputs / stitches outputs.
  - Binarized weights are exactly +-1 in every dtype used here.
  - Mixed-precision contraction: of the 32 k-blocks (128 features each),
    H16 are computed in fp16 (1 cycle/moving-row, ~2^-11 x error) and U8
    in fp8-e4m3 pairs with perf_mode=DoubleRow (2 k-blocks per matmul,
    2 MACs/cell/cycle -> ~1.77x the fp16 rate after the +13% DR tax).
    U8=10 gives worst-case rel err ~1.5e-2 (< 2e-2 gate) measured exactly
    against both CPU- and device-generated reference inputs.
  - Host packs x transposed+tiled so the contraction dim lands on SBUF
    partitions; x is the matmul stationary operand (2-/1-byte stationary
    loads hide in the PE weight shadow buffer), weights stream 512- or
    1024-wide into one PSUM f32 bank per 512 outputs.
  - Per core: resident weight slice in SBUF, stream 128-token blocks of
    xT, accumulate over K=4096 in PSUM, add bias on DVE while copying
    PSUM->SBUF, DMA out.
"""

import sys

if "/opt/trn_rl_repo" not in sys.path:
    sys.path.insert(0, "/opt/trn_rl_repo")

import ml_dtypes
import numpy as np

N_TOK, D_IN, D_OUT = 8192, 4096, 4096
TOK_SHARDS, OUT_SHARDS = 2, 4
N_CORES = TOK_SHARDS * OUT_SHARDS
TOK_C = N_TOK // TOK_SHARDS
OUT_C = D_OUT // OUT_SHARDS
MB = TOK_C // 128  # token blocks per core
KB = D_IN // 128  # contraction blocks
NF = 512  # matmul moving free dim (one fp32 PSUM bank)
NB = OUT_C // NF  # PSUM banks per token block

U8 = 10  # k-blocks computed in fp8-e4m3 DoubleRow pairs (the last U8)
G8 = U8 // 2  # DoubleRow pair groups
H16 = KB - U8  # k-blocks computed in fp16

_cached_nc = None


def build_nc():
    import concourse.bacc as bacc
    import concourse.mybir as mybir
    import concourse.tile as tile

    dt = mybir.dt

    nc = bacc.Bacc()
    xf_d = nc.dram_tensor("xf", [MB, 128, H16 * 128], dt.float16, kind="ExternalInput")
    x8_d = nc.dram_tensor("x8", [MB, 128, U8, 128], dt.float8e4, kind="ExternalInput")
    wf_d = nc.dram_tensor("wf", [H16, 128, OUT_C], dt.float16, kind="ExternalInput")
    w8_d = nc.dram_tensor("w8", [G8, 128, 2, OUT_C], dt.float8e4, kind="ExternalInput")
    br_d = nc.dram_tensor("br", [128, OUT_C], dt.float32, kind="ExternalInput")
    out_d = nc.dram_tensor("out", [TOK_C, OUT_C], dt.float32, kind="ExternalOutput")

    # First TRICKLE token-blocks are loaded before the weight stream and
    # their matmuls interleaved per k-block, so the PE computes while
    # weights arrive instead of idling at kernel start.
    TRICKLE = 4

    with tile.TileContext(nc) as tc:
        with (
            tc.tile_pool(name="wts", bufs=1) as wpool,
            tc.tile_pool(name="bias", bufs=1) as bpool,
            tc.tile_pool(name="xin", bufs=max(2, TRICKLE)) as xpool,
            tc.tile_pool(name="outp", bufs=2) as opool,
            tc.tile_pool(name="psum", bufs=8, space="PSUM") as ppool,
        ):

            def load_x(m):
                xf_m = xpool.tile([128, H16 * 128], dt.float16, name=f"xf_{m}", tag="xf")
                nc.sync.dma_start(xf_m[:], xf_d[m])
                x8_m = xpool.tile([128, U8, 128], dt.float8e4, name=f"x8_{m}", tag="x8")
                nc.sync.dma_start(x8_m[:], x8_d[m])
                return xf_m, x8_m

            def alloc_ps(m):
                return [
                    ppool.tile([128, NF], dt.float32, name=f"ps_{m}_{n}", tag="ps")
                    for n in range(NB)
                ]

            def emit_f16(kb, xf_m, ps):
                # kb in [0, H16): fp16 k-block
                lhs = xf_m[:, kb * 128 : (kb + 1) * 128]
                for n in range(NB):
                    rhs = wfs[kb][:, n * NF : (n + 1) * NF]
                    nc.tensor.matmul(
                        ps[n][:],
                        lhs,
                        rhs,
                        start=False,
                        stop=(kb == H16 - 1),
                    )

            def emit_f8(g, x8_m, ps):
                # g in [0, G8): fp8 DoubleRow pair (two k-blocks per matmul)
                lhs = x8_m[:, 2 * g : 2 * g + 2, :]
                for n in range(NB):
                    rhs = w8s[g][:, :, n * NF : (n + 1) * NF]
                    nc.tensor.matmul(
                        ps[n][:],
                        lhs,
                        rhs,
                        start=(g == 0),
                        stop=False,
                        perf_mode=mybir.MatmulPerfMode.DoubleRow,
                    )

            def emit_mblock(xf_m, x8_m, ps):
                for g in range(G8):
                    emit_f8(g, x8_m, ps)
                for kb in range(H16):
                    emit_f16(kb, xf_m, ps)

            def flush(m, ps):
                out_t = opool.tile([128, OUT_C], dt.float32, name=f"o_{m}", tag="out")
                for n in range(NB):
                    nc.vector.tensor_tensor(
                        out_t[:, n * NF : (n + 1) * NF],
                        ps[n][:],
                        bias_s[:, n * NF : (n + 1) * NF],
                        mybir.AluOpType.add,
                    )
                nc.sync.dma_start(out_d[m * 128 : (m + 1) * 128, :], out_t[:])

            def load_w8(g):
                w = wpool.tile([128, 2, OUT_C], dt.float8e4, name=f"w8_{g}", tag=f"w8_{g}")
                nc.sync.dma_start(w[:], w8_d[g])
                w8s.append(w)

            def load_wf(kb):
                w = wpool.tile([128, OUT_C], dt.float16, name=f"wf{kb}", tag=f"wf{kb}")
                nc.sync.dma_start(w[:], wf_d[kb])
                wfs.append(w)

            # Interleave trickle-x loads with the weight stream so both the
            # PE's first operands and the early k-blocks arrive ASAP.
            wfs, w8s = [], []
            trickle_x = {}
            trickle_x[0] = load_x(0)
            for g in range(G8):
                load_w8(g)
            for kb in range(0, 4):
                load_wf(kb)
            trickle_x[1] = load_x(1)
            for kb in range(4, 12):
                load_wf(kb)
            trickle_x[2] = load_x(2)
            for kb in range(12, H16):
                load_wf(kb)
            bias_s = bpool.tile([128, OUT_C], dt.float32, name="bias_s")
            nc.sync.dma_start(bias_s[:], br_d[:])
            for m in range(3, TRICKLE):
                trickle_x[m] = load_x(m)

            trickle_ps = {m: alloc_ps(m) for m in range(TRICKLE)}
            # m-major chunks ordered to match DMA arrivals so the PE never
            # waits on a late tile. Work units per m-block: fp8 groups
            # 0..G8-1 then fp16 kbs 0..H16-1; unit index u: u<G8 -> fp8
            # group u, else fp16 kb u-G8.
            NU = G8 + H16

            def emit_units(m, u0, u1):
                xf_m, x8_m = trickle_x[m]
                for u in range(u0, u1):
                    if u < G8:
                        emit_f8(u, x8_m, trickle_ps[m])
                    else:
                        emit_f16(u - G8, xf_m, trickle_ps[m])

            C1 = G8 + 4  # units available after phase-1 weight loads
            C2 = G8 + 12
            sched = [
                (0, 0, C1),
                (1, 0, C1),
                (0, C1, C2),
                (1, C1, C2),
                (2, 0, C2),
                (0, C2, NU),
                (1, C2, NU),
                (2, C2, NU),
            ] + [(m, 0, NU) for m in range(3, TRICKLE)]
            for m, u0, u1 in sched:
                emit_units(m, u0, u1)
            for m in range(TRICKLE):
                flush(m, trickle_ps[m])

            for m in range(TRICKLE, MB):
                xf_m, x8_m = load_x(m)
                ps = alloc_ps(m)
                for g in range(G8):
                    emit_f8(g, x8_m, ps)
                for kb in range(H16):
                    emit_f16(kb, xf_m, ps)
                flush(m, ps)

    nc.compile()
    return nc


def _pack_x(a):
    """[TOK_C, nk*128] -> [MB, 128, nk*128] with layout [m, p, (kb t)]:
    packed[m, p, kb*128 + t] = a[m*128 + t, kb*128 + p]."""
    nk = a.shape[1] // 128
    return np.ascontiguousarray(
        a.reshape(MB, 128, nk, 128).transpose(0, 3, 2, 1)
    ).reshape(MB, 128, nk * 128)


def prepare_in_maps(x, weight, bias):
    x = np.asarray(x, dtype=np.float32)
    weight = np.asarray(weight, dtype=np.float32)
    bias = np.asarray(bias, dtype=np.float32)
    E4 = ml_dtypes.float8_e4m3
    KS = H16 * 128  # feature split point

    bw16 = np.where(weight >= 0, np.float16(1.0), np.float16(-1.0))

    wf_packs, w8_packs, bias_packs = [], [], []
    for oi in range(OUT_SHARDS):
        w_sh = bw16[oi * OUT_C : (oi + 1) * OUT_C]  # [OUT_C, D_IN]
        wt = np.ascontiguousarray(w_sh.T)  # [D_IN, OUT_C] fp16
        wf_packs.append(np.ascontiguousarray(wt[:KS].reshape(H16, 128, OUT_C)))
        # [G8, 128, 2, OUT_C]: pair g covers k-blocks (H16+2g, H16+2g+1)
        w8 = wt[KS:].astype(E4).reshape(G8, 2, 128, OUT_C).transpose(0, 2, 1, 3)
        w8_packs.append(np.ascontiguousarray(w8))
        bias_packs.append(
            np.ascontiguousarray(
                np.broadcast_to(bias[oi * OUT_C : (oi + 1) * OUT_C], (128, OUT_C))
            )
        )

    xf_packs, x8_packs = [], []
    for ti in range(TOK_SHARDS):
        x_sh = x[ti * TOK_C : (ti + 1) * TOK_C]
        xf_packs.append(_pack_x(x_sh[:, :KS].astype(np.float16)))
        # [MB, 128, U8, 128]: x8[m, p, j, t] = e4m3(x[m*128+t, KS + j*128 + p])
        x8 = x_sh[:, KS:].astype(E4)  # [TOK_C, U8*128]
        x8 = x8.reshape(MB, 128, U8, 128).transpose(0, 3, 2, 1)
        x8_packs.append(np.ascontiguousarray(x8))

    in_maps = []
    for c in range(N_CORES):
        ti, oi = divmod(c, OUT_SHARDS)
        m = {
            "xf": xf_packs[ti],
            "x8": x8_packs[ti],
            "wf": wf_packs[oi],
            "w8": w8_packs[oi],
            "br": bias_packs[oi],
        }
        in_maps.append(m)
    return in_maps


def run(in_maps, trace=False, **kwargs):
    global _cached_nc
    from concourse.bass_utils import run_bass_kernel_spmd

    if _cached_nc is None:
        _cached_nc = build_nc()
    return run_bass_kernel_spmd(
        _cached_nc, in_maps, list(range(N_CORES)), trace=trace, **kwargs
    )


def gather(results):
    out = np.empty((N_TOK, D_OUT), dtype=np.float32)
    for c in range(N_CORES):
        ti, oi = divmod(c, OUT_SHARDS)
        out[ti * TOK_C : (ti + 1) * TOK_C, oi * OUT_C : (oi + 1) * OUT_C] = results[c][
            "out"
        ]
    return out


def kernel(x, weight, bias):
    res = run(prepare_in_maps(x, weight, bias), trace=False)
    return gather(res.results)


# revision 4
# speedup vs baseline: 1.3509x; 1.0326x over previous
"""Trainium2 kernel for BinaryLinear: out = x @ sign(clip(weight,-1,1)).T + bias.

Full shapes: x [8192, 4096] f32, weight [4096, 4096] f32, bias [4096] f32,
out [8192, 4096] f32.

Strategy (8 NeuronCores, no collectives needed):
  - Grid-shard tokens x out_features across the 8 cores (2x4); each core
    computes a disjoint output tile, host slices inputs / stitches outputs.
  - Binarized weights are exactly +-1 in every dtype used here.
  - Mixed-precision contraction: of the 32 k-blocks (128 features each),
    H16 are computed in fp16 (1 cycle/moving-row, ~2^-11 x error) and U8
    in fp8-e4m3 pairs with perf_mode=DoubleRow (2 k-blocks per matmul at
    the same 512-cycle stream -> 2x rate). U8=12 gives worst-case rel err
    ~1.7e-2 (< 2e-2 gate) measured exactly against both CPU- and
    device-generated reference inputs.
  - Each m-block's matmul sequence opens with an fp16 k-block: its 97 ns
    stationary load hides anywhere, and the 213 ns DoubleRow stationary
    loads that follow hide behind running 213 ns matmuls.
  - Host packs x transposed+tiled so the contraction dim lands on SBUF
    partitions; x is the matmul stationary operand, weights stream 512-
    or 1024-wide into one PSUM f32 bank per 512 outputs.
  - Per core: resident weight slice in SBUF, stream 128-token blocks of
    xT, accumulate over K=4096 in PSUM, add bias on DVE while copying
    PSUM->SBUF, DMA out.
"""

import sys

if "/opt/trn_rl_repo" not in sys.path:
    sys.path.insert(0, "/opt/trn_rl_repo")

import ml_dtypes
import numpy as np

N_TOK, D_IN, D_OUT = 8192, 4096, 4096
TOK_SHARDS, OUT_SHARDS = 2, 4
N_CORES = TOK_SHARDS * OUT_SHARDS
TOK_C = N_TOK // TOK_SHARDS
OUT_C = D_OUT // OUT_SHARDS
MB = TOK_C // 128  # token blocks per core
KB = D_IN // 128  # contraction blocks
NF = 512  # matmul moving free dim (one fp32 PSUM bank)
NB = OUT_C // NF  # PSUM banks per token block

U8 = 12  # k-blocks computed in fp8-e4m3 DoubleRow pairs (the last U8)
G8 = U8 // 2  # DoubleRow pair groups
H16 = KB - U8  # k-blocks computed in fp16

_cached_nc = None


def build_nc():
    import concourse.bacc as bacc
    import concourse.mybir as mybir
    import concourse.tile as tile

    dt = mybir.dt

    nc = bacc.Bacc()
    xf_d = nc.dram_tensor("xf", [MB, 128, H16 * 128], dt.float16, kind="ExternalInput")
    x8_d = nc.dram_tensor("x8", [MB, 128, U8, 128], dt.float8e4, kind="ExternalInput")
    wf_d = nc.dram_tensor("wf", [H16, 128, OUT_C], dt.float16, kind="ExternalInput")
    w8_d = nc.dram_tensor("w8", [G8, 128, 2, OUT_C], dt.float8e4, kind="ExternalInput")
    br_d = nc.dram_tensor("br", [128, OUT_C], dt.float32, kind="ExternalInput")
    out_d = nc.dram_tensor("out", [TOK_C, OUT_C], dt.float32, kind="ExternalOutput")

    # First TRICKLE token-blocks are loaded before the weight stream and
    # their matmuls interleaved per weight arrival, so the PE computes
    # while weights stream in instead of idling at kernel start.
    TRICKLE = 4
    # The very first token-block's fp16 x ships in two chunks so the
    # opening matmul only waits on a 32 KB transfer.
    XC0 = 1  # k-blocks in the first chunk

    with tile.TileContext(nc) as tc:
        with (
            tc.tile_pool(name="wts", bufs=1) as wpool,
            tc.tile_pool(name="bias", bufs=1) as bpool,
            tc.tile_pool(name="xin", bufs=max(2, TRICKLE)) as xpool,
            tc.tile_pool(name="outp", bufs=2) as opool,
            tc.tile_pool(name="psum", bufs=8, space="PSUM") as ppool,
        ):

            def load_x(m, split=False):
                """Returns (fp16 chunk tiles, fp8 tile). Chunk c covers
                k-blocks [cuts[c], cuts[c+1])."""
                if split:
                    cuts = [0, XC0, H16]
                else:
                    cuts = [0, H16]
                chunks = []
                for c in range(len(cuts) - 1):
                    k0, k1 = cuts[c], cuts[c + 1]
                    t = xpool.tile(
                        [128, (k1 - k0) * 128], dt.float16,
                        name=f"xf_{m}_{c}", tag=f"xf{c}",
                    )
                    nc.sync.dma_start(t[:], xf_d[m, :, k0 * 128 : k1 * 128])
                    chunks.append((k0, k1, t))
                x8_m = xpool.tile([128, U8, 128], dt.float8e4, name=f"x8_{m}", tag="x8")
                nc.sync.dma_start(x8_m[:], x8_d[m])
                return chunks, x8_m

            def alloc_ps(m):
                return [
                    ppool.tile([128, NF], dt.float32, name=f"ps_{m}_{n}", tag="ps")
                    for n in range(NB)
                ]

            def emit_f16(kb, chunks, ps, start=False):
                for k0, k1, t in chunks:
                    if k0 <= kb < k1:
                        lhs = t[:, (kb - k0) * 128 : (kb - k0 + 1) * 128]
                        break
                for n in range(NB):
                    rhs = wfs[kb][:, n * NF : (n + 1) * NF]
                    nc.tensor.matmul(
                        ps[n][:],
                        lhs,
                        rhs,
                        start=start,
                        stop=(kb == H16 - 1),
                    )

            def emit_f8(g, x8_m, ps):
                lhs = x8_m[:, 2 * g : 2 * g + 2, :]
                for n in range(NB):
                    rhs = w8s[g][:, :, n * NF : (n + 1) * NF]
                    nc.tensor.matmul(
                        ps[n][:],
                        lhs,
                        rhs,
                        start=False,
                        stop=False,
                        perf_mode=mybir.MatmulPerfMode.DoubleRow,
                    )

            def flush(m, ps):
                out_t = opool.tile([128, OUT_C], dt.float32, name=f"o_{m}", tag="out")
                for n in range(NB):
                    nc.vector.tensor_tensor(
                        out_t[:, n * NF : (n + 1) * NF],
                        ps[n][:],
                        bias_s[:, n * NF : (n + 1) * NF],
                        mybir.AluOpType.add,
                    )
                nc.sync.dma_start(out_d[m * 128 : (m + 1) * 128, :], out_t[:])

            def load_w8(g):
                w = wpool.tile([128, 2, OUT_C], dt.float8e4, name=f"w8_{g}", tag=f"w8_{g}")
                nc.sync.dma_start(w[:], w8_d[g])
                w8s.append(w)

            def load_wf(kb):
                w = wpool.tile([128, OUT_C], dt.float16, name=f"wf{kb}", tag=f"wf{kb}")
                nc.sync.dma_start(w[:], wf_d[kb])
                wfs.append(w)

            # Unit u of an m-block: u=0 -> fp16 kb0 (start=True), u in
            # [1, G8] -> DoubleRow group u-1, u > G8 -> fp16 kb u-G8.
            NU = 1 + G8 + (H16 - 1)

            def emit_units(m, u0, u1):
                chunks, x8_m = trickle_x[m]
                ps = trickle_ps[m]
                for u in range(u0, u1):
                    if u == 0:
                        emit_f16(0, chunks, ps, start=True)
                    elif u <= G8:
                        emit_f8(u - 1, x8_m, ps)
                    else:
                        emit_f16(u - G8, chunks, ps)

            # DMA order: first-chunk of x0 + wf[0] unblock the opening
            # matmul immediately; then x8_0 + w8 groups; then the rest.
            trickle_x = {}
            wfs, w8s = [], []
            c0, x8_0 = load_x(0, split=True)
            trickle_x[0] = (c0, x8_0)
            load_wf(0)
            for g in range(G8):
                load_w8(g)
            for kb in range(1, 4):
                load_wf(kb)
            trickle_x[1] = load_x(1)
            for kb in range(4, 12):
                load_wf(kb)
            trickle_x[2] = load_x(2)
            for kb in range(12, H16):
                load_wf(kb)
            bias_s = bpool.tile([128, OUT_C], dt.float32, name="bias_s")
            nc.sync.dma_start(bias_s[:], br_d[:])
            for m in range(3, TRICKLE):
                trickle_x[m] = load_x(m)

            trickle_ps = {m: alloc_ps(m) for m in range(TRICKLE)}
            C1 = 1 + G8 + 3  # units runnable after wf[0..3] + w8[*]
            C2 = C1 + 8  # + wf[4..11]
            sched = [
                (0, 0, C1),
                (1, 0, C1),
                (0, C1, C2),
                (1, C1, C2),
                (2, 0, C2),
                (0, C2, NU),
                (1, C2, NU),
                (2, C2, NU),
            ] + [(m, 0, NU) for m in range(3, TRICKLE)]
            for m, u0, u1 in sched:
                emit_units(m, u0, u1)
            for m in range(TRICKLE):
                flush(m, trickle_ps[m])

            for m in range(TRICKLE, MB):
                chunks, x8_m = load_x(m)
                ps = alloc_ps(m)
                emit_f16(0, chunks, ps, start=True)
                for g in range(G8):
                    emit_f8(g, x8_m, ps)
                for kb in range(1, H16):
                    emit_f16(kb, chunks, ps)
                flush(m, ps)

    nc.compile()
    return nc


def _pack_x(a):
    """[TOK_C, nk*128] -> [MB, 128, nk*128] with layout [m, p, (kb t)]:
    packed[m, p, kb*128 + t] = a[m*128 + t, kb*128 + p]."""
    nk = a.shape[1] // 128
    return np.ascontiguousarray(
        a.reshape(MB, 128, nk, 128).transpose(0, 3, 2, 1)
    ).reshape(MB, 128, nk * 128)


def prepare_in_maps(x, weight, bias):
    x = np.asarray(x, dtype=np.float32)
    weight = np.asarray(weight, dtype=np.float32)
    bias = np.asarray(bias, dtype=np.float32)
    E4 = ml_dtypes.float8_e4m3
    KS = H16 * 128  # feature split point

    bw16 = np.where(weight >= 0, np.float16(1.0), np.float16(-1.0))

    wf_packs, w8_packs, bias_packs = [], [], []
    for oi in range(OUT_SHARDS):
        w_sh = bw16[oi * OUT_C : (oi + 1) * OUT_C]  # [OUT_C, D_IN]
        wt = np.ascontiguousarray(w_sh.T)  # [D_IN, OUT_C] fp16
        wf_packs.append(np.ascontiguousarray(wt[:KS].reshape(H16, 128, OUT_C)))
        # [G8, 128, 2, OUT_C]: pair g covers k-blocks (H16+2g, H16+2g+1)
        w8 = wt[KS:].astype(E4).reshape(G8, 2, 128, OUT_C).transpose(0, 2, 1, 3)
        w8_packs.append(np.ascontiguousarray(w8))
        bias_packs.append(
            np.ascontiguousarray(
                np.broadcast_to(bias[oi * OUT_C : (oi + 1) * OUT_C], (128, OUT_C))
            )
        )

    xf_packs, x8_packs = [], []
    for ti in range(TOK_SHARDS):
        x_sh = x[ti * TOK_C : (ti + 1) * TOK_C]
        xf_packs.append(_pack_x(x_sh[:, :KS].astype(np.float16)))
        # [MB, 128, U8, 128]: x8[m, p, j, t] = e4m3(x[m*128+t, KS + j*128 + p])
        x8 = x_sh[:, KS:].astype(E4)  # [TOK_C, U8*128]
        x8 = x8.reshape(MB, 128, U8, 128).transpose(0, 3, 2, 1)
        x8_packs.append(np.ascontiguousarray(x8))

    in_maps = []
    for c in range(N_CORES):
        ti, oi = divmod(c, OUT_SHARDS)
        m = {
            "xf": xf_packs[ti],
            "x8": x8_packs[ti],
            "wf": wf_packs[oi],
            "w8": w8_packs[oi],
            "br": bias_packs[oi],
        }
        in_maps.append(m)
    return in_maps


def run(in_maps, trace=False, **kwargs):
    global _cached_nc
    from concourse.bass_utils import run_bass_kernel_spmd

    if _cached_nc is None:
        _cached_nc = build_nc()
    return run_bass_kernel_spmd(
        _cached_nc, in_maps, list(range(N_CORES)), trace=trace, **kwargs
    )


def gather(results):
    out = np.empty((N_TOK, D_OUT), dtype=np.float32)
    for c in range(N_CORES):
        ti, oi = divmod(c, OUT_SHARDS)
        out[ti * TOK_C : (ti + 1) * TOK_C, oi * OUT_C : (oi + 1) * OUT_C] = results[c][
            "out"
        ]
    return out


def kernel(x, weight, bias):
    res = run(prepare_in_maps(x, weight, bias), trace=False)
    return gather(res.results)


# revision 5
# speedup vs baseline: 1.3666x; 1.0116x over previous
"""Trainium2 kernel for BinaryLinear: out = x @ sign(clip(weight,-1,1)).T + bias.

Full shapes: x [8192, 4096] f32, weight [4096, 4096] f32, bias [4096] f32,
out [8192, 4096] f32.

Strategy (8 NeuronCores, no collectives needed):
  - Grid-shard tokens x out_features across the 8 cores (2x4); each core
    computes a disjoint output tile, host slices inputs / stitches outputs.
  - Binarized weights are exactly +-1 in every dtype used here.
  - Mixed-precision contraction: of the 32 k-blocks (128 features each),
    H16 are computed in fp16 (1 cycle/moving-row, ~2^-11 x error) and U8
    in fp8-e4m3 pairs with perf_mode=DoubleRow (2 k-blocks per matmul at
    the same 512-cycle stream -> 2x rate). U8=12 gives worst-case rel err
    ~1.7e-2 (< 2e-2 gate) measured exactly against both CPU- and
    device-generated reference inputs.
  - Each m-block's matmul sequence opens with an fp16 k-block: its 97 ns
    stationary load hides anywhere, and the 213 ns DoubleRow stationary
    loads that follow hide behind running 213 ns matmuls.
  - Host packs x transposed+tiled so the contraction dim lands on SBUF
    partitions; x is the matmul stationary operand, weights stream 512-
    or 1024-wide into one PSUM f32 bank per 512 outputs.
  - Per core: resident weight slice in SBUF, stream 128-token blocks of
    xT, accumulate over K=4096 in PSUM, add bias on DVE while copying
    PSUM->SBUF, DMA out.
"""

import sys

if "/opt/trn_rl_repo" not in sys.path:
    sys.path.insert(0, "/opt/trn_rl_repo")

import ml_dtypes
import numpy as np

N_TOK, D_IN, D_OUT = 8192, 4096, 4096
TOK_SHARDS, OUT_SHARDS = 2, 4
N_CORES = TOK_SHARDS * OUT_SHARDS
TOK_C = N_TOK // TOK_SHARDS
OUT_C = D_OUT // OUT_SHARDS
MB = TOK_C // 128  # token blocks per core
KB = D_IN // 128  # contraction blocks
NF = 512  # matmul moving free dim (one fp32 PSUM bank)
NB = OUT_C // NF  # PSUM banks per token block

U8 = 12  # k-blocks computed in fp8-e4m3 DoubleRow pairs (the last U8)
G8 = U8 // 2  # DoubleRow pair groups
H16 = KB - U8  # k-blocks computed in fp16

_cached_nc = None


def build_nc():
    import concourse.bacc as bacc
    import concourse.mybir as mybir
    import concourse.tile as tile

    dt = mybir.dt

    nc = bacc.Bacc()
    xf_d = nc.dram_tensor("xf", [MB, 128, H16 * 128], dt.float16, kind="ExternalInput")
    x8_d = nc.dram_tensor("x8", [MB, 128, U8, 128], dt.float8e4, kind="ExternalInput")
    wf_d = nc.dram_tensor("wf", [H16, 128, OUT_C], dt.float16, kind="ExternalInput")
    w8_d = nc.dram_tensor("w8", [G8, 128, 2, OUT_C], dt.float8e4, kind="ExternalInput")
    br_d = nc.dram_tensor("br", [128, OUT_C], dt.float32, kind="ExternalInput")
    out_d = nc.dram_tensor("out", [TOK_C, OUT_C], dt.float32, kind="ExternalOutput")

    # First TRICKLE token-blocks are loaded before the weight stream and
    # their matmuls interleaved per weight arrival, so the PE computes
    # while weights stream in instead of idling at kernel start. The fp8
    # operands go first: x8 tiles (196 KB) plus the six w8 groups give the
    # PE ~5 us of DoubleRow runway for the price of ~2 MB of DMA.
    TRICKLE = 4

    with tile.TileContext(nc) as tc:
        with (
            tc.tile_pool(name="wts", bufs=1) as wpool,
            tc.tile_pool(name="bias", bufs=1) as bpool,
            tc.tile_pool(name="xin", bufs=max(2, TRICKLE)) as xpool,
            tc.tile_pool(name="outp", bufs=2) as opool,
            tc.tile_pool(name="psum", bufs=8, space="PSUM") as ppool,
        ):

            def load_x8(m):
                x8_m = xpool.tile([128, U8, 128], dt.float8e4, name=f"x8_{m}", tag="x8")
                nc.sync.dma_start(x8_m[:], x8_d[m])
                return x8_m

            def load_xf(m):
                xf_m = xpool.tile([128, H16 * 128], dt.float16, name=f"xf_{m}", tag="xf")
                nc.sync.dma_start(xf_m[:], xf_d[m])
                return xf_m

            def load_x(m):
                return load_xf(m), load_x8(m)

            def alloc_ps(m):
                return [
                    ppool.tile([128, NF], dt.float32, name=f"ps_{m}_{n}", tag="ps")
                    for n in range(NB)
                ]

            def emit_f16(kb, xf_m, ps):
                lhs = xf_m[:, kb * 128 : (kb + 1) * 128]
                for n in range(NB):
                    rhs = wfs[kb][:, n * NF : (n + 1) * NF]
                    nc.tensor.matmul(
                        ps[n][:],
                        lhs,
                        rhs,
                        start=False,
                        stop=(kb == H16 - 1),
                    )

            def emit_f8(g, x8_m, ps):
                lhs = x8_m[:, 2 * g : 2 * g + 2, :]
                for n in range(NB):
                    rhs = w8s[g][:, :, n * NF : (n + 1) * NF]
                    nc.tensor.matmul(
                        ps[n][:],
                        lhs,
                        rhs,
                        start=(g == 0),
                        stop=False,
                        perf_mode=mybir.MatmulPerfMode.DoubleRow,
                    )

            def flush(m, ps, split=False):
                out_t = opool.tile([128, OUT_C], dt.float32, name=f"o_{m}", tag="out")
                for n in range(NB):
                    nc.vector.tensor_tensor(
                        out_t[:, n * NF : (n + 1) * NF],
                        ps[n][:],
                        bias_s[:, n * NF : (n + 1) * NF],
                        mybir.AluOpType.add,
                    )
                    if split:
                        # last block: per-bank DMA shortens the serial tail
                        nc.sync.dma_start(
                            out_d[m * 128 : (m + 1) * 128, n * NF : (n + 1) * NF],
                            out_t[:, n * NF : (n + 1) * NF],
                        )
                if not split:
                    nc.sync.dma_start(out_d[m * 128 : (m + 1) * 128, :], out_t[:])

            def load_w8(g):
                w = wpool.tile([128, 2, OUT_C], dt.float8e4, name=f"w8_{g}", tag=f"w8_{g}")
                nc.sync.dma_start(w[:], w8_d[g])
                w8s.append(w)

            def load_wf(kb):
                w = wpool.tile([128, OUT_C], dt.float16, name=f"wf{kb}", tag=f"wf{kb}")
                nc.sync.dma_start(w[:], wf_d[kb])
                wfs.append(w)

            # Unit u of an m-block: u < G8 -> DoubleRow group u (start on
            # g0), u >= G8 -> fp16 k-block u-G8 (stop on the last).
            NU = G8 + H16

            def emit_units(m, u0, u1):
                xf_m, x8_m = trickle_x[m]
                ps = trickle_ps[m]
                for u in range(u0, u1):
                    if u < G8:
                        emit_f8(u, x8_m, ps)
                    else:
                        emit_f16(u - G8, xf_m, ps)

            # DMA issue order tracks the emission schedule below: fp8
            # operands for m0/m1 first (cheap bytes, 24 matmuls of runway),
            # then the fp16 weight stream with x tiles slotted into the
            # PE's surplus.
            trickle_x = {m: [None, None] for m in range(TRICKLE)}
            wfs, w8s = [], []
            trickle_x[0][1] = load_x8(0)
            load_w8(0)
            trickle_x[1][1] = load_x8(1)
            for g in range(1, G8):
                load_w8(g)
            trickle_x[0][0] = load_xf(0)
            load_wf(0)
            trickle_x[1][0] = load_xf(1)
            load_wf(1)
            load_wf(2)
            trickle_x[2][1] = load_x8(2)
            trickle_x[2][0] = load_xf(2)
            load_wf(3)
            load_wf(4)
            trickle_x[3][1] = load_x8(3)
            trickle_x[3][0] = load_xf(3)
            for kb in range(5, H16):
                load_wf(kb)
            bias_s = bpool.tile([128, OUT_C], dt.float32, name="bias_s")
            nc.sync.dma_start(bias_s[:], br_d[:])

            trickle_ps = {m: alloc_ps(m) for m in range(TRICKLE)}
            sched = [
                (0, 0, 1), (1, 0, 1),
                (0, 1, 2), (1, 1, 2),
                (0, 2, 4), (1, 2, 4),
                (0, 4, G8), (1, 4, G8),
                (0, G8, G8 + 1), (1, G8, G8 + 1),      # kb0
                (0, G8 + 1, G8 + 2), (1, G8 + 1, G8 + 2),
                (2, 0, G8),                             # m2 fp8 burst
                (0, G8 + 2, G8 + 4), (1, G8 + 2, G8 + 4),
                (2, G8, G8 + 2),
                (3, 0, G8),                             # m3 fp8 burst
                (0, G8 + 4, G8 + 5), (1, G8 + 4, G8 + 5),
                (2, G8 + 2, G8 + 4), (3, G8, G8 + 2),
                (0, G8 + 5, NU), (1, G8 + 5, NU),
                (2, G8 + 4, NU), (3, G8 + 2, NU),
            ]
            for m, u0, u1 in sched:
                emit_units(m, u0, u1)
            for m in range(TRICKLE):
                flush(m, trickle_ps[m])

            for m in range(TRICKLE, MB):
                xf_m, x8_m = load_x(m)
                ps = alloc_ps(m)
                for g in range(G8):
                    emit_f8(g, x8_m, ps)
                for kb in range(H16):
                    emit_f16(kb, xf_m, ps)
                flush(m, ps, split=(m == MB - 1))

    nc.compile()
    return nc


def _pack_x(a):
    """[TOK_C, nk*128] -> [MB, 128, nk*128] with layout [m, p, (kb t)]:
    packed[m, p, kb*128 + t] = a[m*128 + t, kb*128 + p]."""
    nk = a.shape[1] // 128
    return np.ascontiguousarray(
        a.reshape(MB, 128, nk, 128).transpose(0, 3, 2, 1)
    ).reshape(MB, 128, nk * 128)


def prepare_in_maps(x, weight, bias):
    x = np.asarray(x, dtype=np.float32)
    weight = np.asarray(weight, dtype=np.float32)
    bias = np.asarray(bias, dtype=np.float32)
    E4 = ml_dtypes.float8_e4m3
    KS = H16 * 128  # feature split point

    bw16 = np.where(weight >= 0, np.float16(1.0), np.float16(-1.0))

    wf_packs, w8_packs, bias_packs = [], [], []
    for oi in range(OUT_SHARDS):
        w_sh = bw16[oi * OUT_C : (oi + 1) * OUT_C]  # [OUT_C, D_IN]
        wt = np.ascontiguousarray(w_sh.T)  # [D_IN, OUT_C] fp16
        wf_packs.append(np.ascontiguousarray(wt[:KS].reshape(H16, 128, OUT_C)))
        # [G8, 128, 2, OUT_C]: pair g covers k-blocks (H16+2g, H16+2g+1)
        w8 = wt[KS:].astype(E4).reshape(G8, 2, 128, OUT_C).transpose(0, 2, 1, 3)
        w8_packs.append(np.ascontiguousarray(w8))
        bias_packs.append(
            np.ascontiguousarray(
                np.broadcast_to(bias[oi * OUT_C : (oi + 1) * OUT_C], (128, OUT_C))
            )
        )

    xf_packs, x8_packs = [], []
    for ti in range(TOK_SHARDS):
        x_sh = x[ti * TOK_C : (ti + 1) * TOK_C]
        xf_packs.append(_pack_x(x_sh[:, :KS].astype(np.float16)))
        # [MB, 128, U8, 128]: x8[m, p, j, t] = e4m3(x[m*128+t, KS + j*128 + p])
        x8 = x_sh[:, KS:].astype(E4)  # [TOK_C, U8*128]
        x8 = x8.reshape(MB, 128, U8, 128).transpose(0, 3, 2, 1)
        x8_packs.append(np.ascontiguousarray(x8))

    in_maps = []
    for c in range(N_CORES):
        ti, oi = divmod(c, OUT_SHARDS)
        m = {
            "xf": xf_packs[ti],
            "x8": x8_packs[ti],
            "wf": wf_packs[oi],
            "w8": w8_packs[oi],
            "br": bias_packs[oi],
        }
        in_maps.append(m)
    return in_maps


def run(in_maps, trace=False, **kwargs):
    global _cached_nc
    from concourse.bass_utils import run_bass_kernel_spmd

    if _cached_nc is None:
        _cached_nc = build_nc()
    return run_bass_kernel_spmd(
        _cached_nc, in_maps, list(range(N_CORES)), trace=trace, **kwargs
    )


def gather(results):
    out = np.empty((N_TOK, D_OUT), dtype=np.float32)
    for c in range(N_CORES):
        ti, oi = divmod(c, OUT_SHARDS)
        out[ti * TOK_C : (ti + 1) * TOK_C, oi * OUT_C : (oi + 1) * OUT_C] = results[c][
            "out"
        ]
    return out


def kernel(x, weight, bias):
    res = run(prepare_in_maps(x, weight, bias), trace=False)
    return gather(res.results)
